# revision 13
# baseline (speedup 1.0000x reference)
"""Trainium2 Bass kernel for the dense_mlp problem (8 NeuronCores, data parallel).

Network: x[N,2] -> Linear(2,16)+tanh -> 8x (Linear(16,16)+tanh)
         -> Linear(16,3)+sigmoid, N = 4_194_304.

Strategy (per core, N_shard = 524288 pixels):
  - 8 pixel-streams of 65536 pixels; activations kept transposed in SBUF
    as [128, cols] tiles (partition 16*s+j = feature j of stream s).
  - Weights block-diagonal [128,128] (8 copies of W.T), so each matmul
    column advances 8 pixels. Matmuls in bf16; PSUM accumulates fp32.
  - tanh split across TWO engines running concurrently:
      * ScalarE (ACT): native tanh, handles layer 0 (with bias), the
        sigmoid output, and ~half the hidden chunk-layers.
      * VectorE (DVE): a custom fused DVE op (TANH57_ANT) computing a
        clamped deg-5 odd polynomial c*(k0 + k1 c^2 + k2 c^4),
        c = clamp(x, +-2.4), in ONE instruction pass (1 elem/cycle/lane)
        with per-layer density-fitted coefficients. Handles the other
        half of the hidden chunk-layers.
  - Chunks of 2048 pixels interleaved in quads so TensorE matmuls of one
    chunk overlap ACT/DVE activations of others (2 PSUM buffers).
  - Host does the free layout transposes (shard / interleave).
"""

import itertools

import ml_dtypes
import numpy as np

import concourse.bass as bass
import concourse.mybir as mybir
from concourse import bacc
from concourse.bass_utils import run_bass_kernel_spmd
from concourse.tile import TileContext

N_HID_LAYERS = 8
N_STREAMS = 8
N_CORES = 8
N_COLS = 65536          # pixels per stream per core
CHUNK = 2048
MM_BLOCK = 512
GRP = 2

AF = mybir.ActivationFunctionType
BF16 = ml_dtypes.bfloat16

# --- custom DVE op: clamped deg-5 odd polynomial tanh --------------------
# tanh(x) ~ c*(k0 + t*(k1 + t*k2)), c = clamp(x, +-BC), t = c^2.
# Slots: s0=k0 (per-layer), s1=k1 (per-layer), imm2=BC, in1=[128,1] k2.
# Exactly 8 ALU stages: minn, maxx, sq, *k2, +k1, *t, +k0, *c.
TANH_NAME = "TANH57_ANT"
BC = 2.4
K2 = 0.020211224165645702
KS = [(0.9659010008340659, -0.2068142971910713),
      (0.9695863476358041, -0.21800918949272657),
      (0.9785551732753938, -0.21214304632816955),
      (0.9832893503762081, -0.21140289967047618),
      (0.9879874103093083, -0.21196830047300133),
      (0.9803683425918669, -0.21122284283184908),
      (0.9936387695522303, -0.21297461925998967),
      (0.9918541550187544, -0.21256005611456705)]

# engine assignment for hidden-layer tanh: ScalarE owns layers 0-3 (plus
# layer 0 bias + output sigmoid), VectorE owns layers 4-7. The emission is
# software-pipelined: quad p's tail (l4-l7, DVE) interleaves with quad
# p+1's head (l0-l3, ACT) so both engines stream across quad boundaries.
N_HEAD = 4  # layers 1..4 of the hidden stack handled by ACT (l0 too)


def _tanh_ref(in0, in1, s0, s1, imm2):
    k2 = np.asarray(in1).reshape(np.asarray(in1).shape[0], -1)[:, :1]
    c = np.clip(in0, -imm2, imm2)
    t = c * c
    return (((k2 * t + s1) * t + s0) * c).astype(np.float32)


def _register_tanh_op():
    from concourse import dve_ops
    from concourse.dve_spec import (
        Spec, Src0, C0, C1, C2, C3, Zero, maxx, minn, sq,
        _spill_c3_to_src1, lower,
    )
    from concourse.dve_uop import DveOpSpec

    if TANH_NAME in dve_ops._SUB_OPCODE_FOR_NAME:
        return next(o for o in dve_ops.OPS if o.name == TANH_NAME)
    row = dve_ops._CUSTOM_DVE_ROW_BASE + len(dve_ops.OPS)
    assert row < 0x20
    dve_ops._SUB_OPCODE_FOR_NAME[TANH_NAME] = row
    c = maxx(minn(Src0, C2), Zero - C2)
    t = sq(c)
    body = ((C3 * t + C1) * t + C0) * c
    spec = Spec(body=_spill_c3_to_src1(body), reference=_tanh_ref)
    shas = {}
    for ver in ("v3", "v4"):
        uops = lower(spec, ver=ver)
        shas[ver] = DveOpSpec(name=TANH_NAME, opcode=row, uops=uops,
                              rd1_en=True).sha(ver)
    op = dve_ops.DveOp(TANH_NAME, spec, subdim=False, uops_sha=shas)
    dve_ops.OPS.append(op)
    return op


TANH_OP = _register_tanh_op()

LAST_RUN_INFO = {}

_GRAPH_CACHE = {}


def _build_graph(n_cols, chunk=CHUNK, mm_block=MM_BLOCK):
    key = (n_cols, chunk, mm_block)
    if key in _GRAPH_CACHE:
        return _GRAPH_CACHE[key]
    n_grp = n_cols // (GRP * chunk)
    blocks = chunk // mm_block

    nc = bacc.Bacc()
    f32 = mybir.dt.float32
    mm_dt = mybir.dt.bfloat16

    xT = nc.declare_dram_parameter("xT", [2 * N_STREAMS, n_cols], mm_dt, isOutput=False)
    w0 = nc.declare_dram_parameter("w0", [2 * N_STREAMS, 128], mm_dt, isOutput=False)
    b0 = nc.declare_dram_parameter("b0", [128, 1], f32, isOutput=False)
    wh = nc.declare_dram_parameter("wh", [N_HID_LAYERS, 128, 128], mm_dt, isOutput=False)
    wo = nc.declare_dram_parameter("wo", [128, 3 * N_STREAMS], mm_dt, isOutput=False)
    k2c = nc.declare_dram_parameter("k2c", [128, 1], f32, isOutput=False)
    out = nc.declare_dram_parameter("out", [3 * N_STREAMS, n_cols], f32, isOutput=True)

    half = chunk // 2
    with TileContext(nc) as tc:
        with (
            tc.tile_pool(name="wpool", bufs=1) as wpool,
            tc.tile_pool(name="xpool", bufs=6) as xpool,
            tc.tile_pool(name="hpool", bufs=10) as hpool,
            tc.tile_pool(name="opool", bufs=4) as opool,
            tc.tile_pool(name="psum", bufs=4, space="PSUM") as psum_pool,
        ):
            w0_sb = wpool.tile([2 * N_STREAMS, 128], mm_dt)
            nc.sync.dma_start(out=w0_sb, in_=w0[:, :])
            b0_sb = wpool.tile([128, 1], f32)
            nc.sync.dma_start(out=b0_sb, in_=b0[:, :])
            k2_sb = wpool.tile([128, 1], f32)
            nc.sync.dma_start(out=k2_sb, in_=k2c[:, :])
            wh_sb = []
            for l in range(N_HID_LAYERS):
                w_l = wpool.tile([128, 128], mm_dt, name=f"wh{l}")
                nc.sync.dma_start(out=w_l, in_=wh[l, :, :])
                wh_sb.append(w_l)
            wo_sb = wpool.tile([128, 3 * N_STREAMS], mm_dt)
            nc.sync.dma_start(out=wo_sb, in_=wo[:, :])

            # per-layer unit = half a chunk ([128, 1024] PSUM slot)
            ORDER = tuple(range(GRP))
            SPLIT_L = 3          # hidden layer split between ACT and DVE
            ACT_CHUNKS = (0,)    # chunks of SPLIT_L handled by ACT

            def unit(ps, w_sb, h_in, hb):
                for k in range(2):
                    bs = slice(hb * half + k * mm_block,
                               hb * half + (k + 1) * mm_block)
                    nc.tensor.matmul(ps[:, k * mm_block:(k + 1) * mm_block],
                                     w_sb, h_in[:, bs], start=True, stop=True)

            # software-pipelined state
            x_sb = {}   # (q, i) -> x tile
            h_cur = {}  # (q, i) -> latest h tile

            def emit_x(q):
                for i in range(GRP):
                    cs = slice((GRP * q + i) * chunk, (GRP * q + i + 1) * chunk)
                    t = xpool.tile([2 * N_STREAMS, chunk], mm_dt, tag="x",
                                   name=f"x{i}")
                    nc.sync.dma_start(out=t, in_=xT[:, cs])
                    x_sb[(q, i)] = t

            def hidden_unit(q, l, i, hb, h_next, dve):
                # 32x32 tile-position matmuls: the hidden weights are
                # block-diagonal (8 identical 16x16 blocks), so only the
                # four diagonal 32x32 tiles of the PE array do useful work.
                # Dispatch 8 concurrent tile matmuls (input strips
                # s in {2hb, 2hb+1} x all 4 col-blocks), each streaming its
                # own 512 columns — ~4x PE throughput vs full-array matmul.
                # Output lands at (strip b, block s): an involution, so
                # after the 8 hidden layers pixels are back at their
                # natural positions.
                ps = psum_pool.tile([128, half], f32, tag="ps",
                                    name=f"ps{i}{hb}")
                h_in = h_cur[(q, i)]
                for s in (2 * hb, 2 * hb + 1):
                    w_t = wh_sb[l][32 * s:32 * s + 32, 32 * s:32 * s + 32]
                    sl = (s - 2 * hb) * mm_block
                    for b in range(blocks):
                        nc.tensor.matmul(
                            ps[32 * b:32 * b + 32, sl:sl + mm_block],
                            w_t,
                            h_in[32 * s:32 * s + 32,
                                 b * mm_block:(b + 1) * mm_block],
                            start=True, stop=True,
                            tile_position=(32 * s, 32 * b))
                dst = h_next[:, hb * half:(hb + 1) * half]
                if dve:
                    nc.vector._custom_dve(
                        TANH_OP, out=dst, in0=ps[:, :], in1=k2_sb[:, :],
                        s0=KS[l][0], s1=KS[l][1], imm2=BC)
                else:
                    nc.scalar.activation(dst, ps, AF.Tanh)

            def gen_head(q):
                # input layer (bias) on ACT
                for i in ORDER:
                    t = hpool.tile([128, chunk], mm_dt, tag="h", name=f"h{i}")
                    for hb in range(2):
                        ps = psum_pool.tile([128, half], f32, tag="ps",
                                            name=f"ps{i}{hb}")
                        unit(ps, w0_sb, x_sb.pop((q, i)) if hb else
                             x_sb[(q, i)], hb)
                        nc.scalar.activation(
                            t[:, hb * half:(hb + 1) * half], ps,
                            AF.Tanh, bias=b0_sb)
                        h_cur[(q, i)] = t
                        yield
                # hidden layers 0..SPLIT_L-1 on ACT
                for l in range(SPLIT_L):
                    for i in ORDER:
                        t = hpool.tile([128, chunk], mm_dt, tag="h",
                                       name=f"h{i}")
                        for hb in range(2):
                            hidden_unit(q, l, i, hb, t, dve=False)
                            yield
                        h_cur[(q, i)] = t
                # SPLIT_L for ACT's chunks
                for i in ACT_CHUNKS:
                    t = hpool.tile([128, chunk], mm_dt, tag="h", name=f"h{i}")
                    for hb in range(2):
                        hidden_unit(q, SPLIT_L, i, hb, t, dve=False)
                        yield
                    h_cur[(q, i)] = t

            def gen_tail(q):
                # SPLIT_L for DVE's chunks
                for i in ORDER:
                    if i in ACT_CHUNKS:
                        continue
                    t = hpool.tile([128, chunk], mm_dt, tag="h", name=f"h{i}")
                    for hb in range(2):
                        hidden_unit(q, SPLIT_L, i, hb, t, dve=True)
                        yield
                    h_cur[(q, i)] = t
                # hidden layers SPLIT_L+1..7 on DVE
                for l in range(SPLIT_L + 1, N_HID_LAYERS):
                    for i in ORDER:
                        t = hpool.tile([128, chunk], mm_dt, tag="h",
                                       name=f"h{i}")
                        for hb in range(2):
                            hidden_unit(q, l, i, hb, t, dve=True)
                            yield
                        h_cur[(q, i)] = t

            def gen_out(q):
                o_grp = opool.tile([128, GRP * mm_block], f32, tag="o",
                                   name="o")
                for pair in range(GRP // 2):
                    ps_o = psum_pool.tile([128, half], f32, tag="ps",
                                          name=f"pso{pair}")
                    for ii in range(2):
                        i = 2 * pair + ii
                        hq = h_cur[(q, i)]
                        for k in range(blocks):
                            bs = slice(k * mm_block, (k + 1) * mm_block)
                            os_ = slice(ii * mm_block, (ii + 1) * mm_block)
                            nc.tensor.matmul(
                                ps_o[32 * k:32 * k + 3 * N_STREAMS, os_],
                                wo_sb, hq[:, bs], start=True, stop=True,
                                tile_position=(0, 32 * k))
                        yield
                    nc.scalar.activation(
                        o_grp[:, pair * half:(pair + 1) * half], ps_o,
                        AF.Sigmoid)
                    yield
                for i in range(GRP):
                    del h_cur[(q, i)]
                # o_grp[32k+m, 512i+c] = out[m, (GRP*q+i)*2048 + 512k + c]
                for k in range(blocks):
                    src = o_grp[32 * k:32 * k + 3 * N_STREAMS, :]
                    dst = out.rearrange("m (q i kk c) -> kk m q i c",
                                        q=n_grp, i=GRP, kk=blocks)[k, :, q]
                    nc.sync.dma_start(out=dst, in_=src)
                yield

            emit_x(0)
            for q in range(n_grp + 2):
                if q + 1 < n_grp:
                    emit_x(q + 1)
                # weave tail(q-1) [DVE], head(q) [ACT] and out(q-2)
                # [PE+sigmoid] so no engine starves at group boundaries
                tail_it = gen_tail(q - 1) if 1 <= q < n_grp + 1 else iter(())
                head_it = gen_head(q) if q < n_grp else iter(())
                out_it = gen_out(q - 2) if 2 <= q < n_grp + 2 else iter(())
                weave = itertools.cycle((tail_it, head_it, tail_it, head_it,
                                         out_it))
                live = {id(tail_it): True, id(head_it): True, id(out_it): True}
                done = 0
                while done < 3:
                    it = next(weave)
                    if not live[id(it)]:
                        continue
                    if next(it, StopIteration) is StopIteration:
                        live[id(it)] = False
                        done = sum(1 for v in live.values() if not v)

    nc.compile()
    _GRAPH_CACHE[key] = nc
    return nc


def _pack_weights(W0, b0, Wh, Wout):
    w0p = np.zeros((2 * N_STREAMS, 128), np.float32)
    b0p = np.zeros((128, 1), np.float32)
    whp = np.zeros((N_HID_LAYERS, 128, 128), np.float32)
    wop = np.zeros((128, 3 * N_STREAMS), np.float32)
    for s in range(N_STREAMS):
        w0p[2 * s:2 * s + 2, 16 * s:16 * s + 16] = W0.T
        b0p[16 * s:16 * s + 16, 0] = b0
        for l in range(N_HID_LAYERS):
            whp[l, 16 * s:16 * s + 16, 16 * s:16 * s + 16] = Wh[l].T
        wop[16 * s:16 * s + 16, 3 * s:3 * s + 3] = Wout.T
    return w0p, b0p, whp, wop


def kernel(x, W0, b0, Wh, Wout, trace=False):
    x = np.asarray(x, np.float32)
    W0 = np.asarray(W0, np.float32)
    b0 = np.asarray(b0, np.float32)
    Wh = np.asarray(Wh, np.float32)
    Wout = np.asarray(Wout, np.float32)

    nc = _build_graph(N_COLS)
    w0p, b0p, whp, wop = _pack_weights(W0, b0, Wh, Wout)
    w0p, whp, wop = (a.astype(BF16) for a in (w0p, whp, wop))
    k2arr = np.full((128, 1), K2, np.float32)

    per_core = N_STREAMS * N_COLS
    in_maps = []
    for c in range(N_CORES):
        xs = x[c * per_core:(c + 1) * per_core]
        xT = np.ascontiguousarray(
            xs.reshape(N_STREAMS, N_COLS, 2).transpose(0, 2, 1)
        ).reshape(2 * N_STREAMS, N_COLS)
        in_maps.append({"xT": xT.astype(BF16), "w0": w0p, "b0": b0p,
                        "wh": whp, "wo": wop, "k2c": k2arr})

    res = run_bass_kernel_spmd(nc, in_maps, core_ids=list(range(N_CORES)),
                               trace=trace)
    LAST_RUN_INFO.clear()
    LAST_RUN_INFO["exec_time_ns"] = res.exec_time_ns
    prof = getattr(res, "instructions_and_trace", None)
    LAST_RUN_INFO["trace_dir"] = getattr(prof, "trace_dir", None)

    parts = []
    for r in res.results:
        o = r["out"].reshape(N_STREAMS, 3, N_COLS).transpose(0, 2, 1)
        parts.append(o.reshape(per_core, 3))
    return np.concatenate(parts, axis=0)


# revision 16
# speedup vs baseline: 1.1057x; 1.1057x over previous
"""Trainium2 Bass kernel for the dense_mlp problem (8 NeuronCores, data parallel).

Network: x[N,2] -> Linear(2,16)+tanh -> 8x (Linear(16,16)+tanh)
         -> Linear(16,3)+sigmoid, N = 4_194_304.

Strategy (per core, N_shard = 524288 pixels):
  - 8 pixel-streams of 65536 pixels; activations kept transposed in SBUF
    as [128, cols] tiles (partition 16*s+j = feature j of stream s).
  - Weights block-diagonal [128,128] (8 copies of W.T), so each matmul
    column advances 8 pixels. Matmuls in bf16; PSUM accumulates fp32.
  - tanh split across TWO engines running concurrently:
      * ScalarE (ACT): native tanh, handles layer 0 (with bias), the
        sigmoid output, and ~half the hidden chunk-layers.
      * VectorE (DVE): a custom fused DVE op (TANH57_ANT) computing a
        clamped deg-5 odd polynomial c*(k0 + k1 c^2 + k2 c^4),
        c = clamp(x, +-2.4), in ONE instruction pass (1 elem/cycle/lane)
        with per-layer density-fitted coefficients. Handles the other
        half of the hidden chunk-layers.
  - Chunks of 2048 pixels interleaved in quads so TensorE matmuls of one
    chunk overlap ACT/DVE activations of others (2 PSUM buffers).
  - Host does the free layout transposes (shard / interleave).
"""

import itertools

import ml_dtypes
import numpy as np

import concourse.bass as bass
import concourse.mybir as mybir
from concourse import bacc
from concourse.bass_utils import run_bass_kernel_spmd
from concourse.tile import TileContext

N_HID_LAYERS = 8
N_STREAMS = 8
N_CORES = 8
N_COLS = 65536          # pixels per stream per core
CHUNK = 2048
MM_BLOCK = 512
GRP = 2

AF = mybir.ActivationFunctionType
BF16 = ml_dtypes.bfloat16

# --- custom DVE op: clamped deg-5 odd polynomial tanh --------------------
# tanh(x) ~ c*(k0 + t*(k1 + t*k2)), c = clamp(x, +-BC), t = c^2.
# Slots: s0=k0 (per-layer), s1=k1 (per-layer), imm2=BC, in1=[128,1] k2.
# Exactly 8 ALU stages: minn, maxx, sq, *k2, +k1, *t, +k0, *c.
TANH_NAME = "TANH57_ANT"
BC = 2.4
K2 = 0.020211224165645702
KS = [(0.9659010008340659, -0.2068142971910713),
      (0.9695863476358041, -0.21800918949272657),
      (0.9785551732753938, -0.21214304632816955),
      (0.9832893503762081, -0.21140289967047618),
      (0.9879874103093083, -0.21196830047300133),
      (0.9803683425918669, -0.21122284283184908),
      (0.9936387695522303, -0.21297461925998967),
      (0.9918541550187544, -0.21256005611456705)]

# engine assignment for hidden-layer tanh: ScalarE owns layers 0-3 (plus
# layer 0 bias + output sigmoid), VectorE owns layers 4-7. The emission is
# software-pipelined: quad p's tail (l4-l7, DVE) interleaves with quad
# p+1's head (l0-l3, ACT) so both engines stream across quad boundaries.
N_HEAD = 4  # layers 1..4 of the hidden stack handled by ACT (l0 too)


def _tanh_ref(in0, in1, s0, s1, imm2):
    k2 = np.asarray(in1).reshape(np.asarray(in1).shape[0], -1)[:, :1]
    c = np.clip(in0, -imm2, imm2)
    t = c * c
    return (((k2 * t + s1) * t + s0) * c).astype(np.float32)


def _register_tanh_op():
    from concourse import dve_ops
    from concourse.dve_spec import (
        Spec, Src0, C0, C1, C2, C3, Zero, maxx, minn, sq,
        _spill_c3_to_src1, lower,
    )
    from concourse.dve_uop import DveOpSpec

    if TANH_NAME in dve_ops._SUB_OPCODE_FOR_NAME:
        return next(o for o in dve_ops.OPS if o.name == TANH_NAME)
    row = dve_ops._CUSTOM_DVE_ROW_BASE + len(dve_ops.OPS)
    assert row < 0x20
    dve_ops._SUB_OPCODE_FOR_NAME[TANH_NAME] = row
    c = maxx(minn(Src0, C2), Zero - C2)
    t = sq(c)
    body = ((C3 * t + C1) * t + C0) * c
    spec = Spec(body=_spill_c3_to_src1(body), reference=_tanh_ref)
    shas = {}
    for ver in ("v3", "v4"):
        uops = lower(spec, ver=ver)
        shas[ver] = DveOpSpec(name=TANH_NAME, opcode=row, uops=uops,
                              rd1_en=True).sha(ver)
    op = dve_ops.DveOp(TANH_NAME, spec, subdim=False, uops_sha=shas)
    dve_ops.OPS.append(op)
    return op


TANH_OP = _register_tanh_op()

LAST_RUN_INFO = {}

_GRAPH_CACHE = {}


def _build_graph(n_cols, chunk=CHUNK, mm_block=MM_BLOCK):
    key = (n_cols, chunk, mm_block)
    if key in _GRAPH_CACHE:
        return _GRAPH_CACHE[key]
    n_grp = n_cols // (GRP * chunk)
    blocks = chunk // mm_block

    nc = bacc.Bacc()
    f32 = mybir.dt.float32
    mm_dt = mybir.dt.bfloat16

    xT = nc.declare_dram_parameter("xT", [2 * N_STREAMS, n_cols], mm_dt, isOutput=False)
    w0 = nc.declare_dram_parameter("w0", [2 * N_STREAMS, 128], mm_dt, isOutput=False)
    b0 = nc.declare_dram_parameter("b0", [128, 1], f32, isOutput=False)
    wh = nc.declare_dram_parameter("wh", [N_HID_LAYERS, 128, 128], mm_dt, isOutput=False)
    wo = nc.declare_dram_parameter("wo", [128, 3 * N_STREAMS], mm_dt, isOutput=False)
    k2c = nc.declare_dram_parameter("k2c", [128, 1], f32, isOutput=False)
    out = nc.declare_dram_parameter("out", [3 * N_STREAMS, n_cols], f32, isOutput=True)

    half = chunk // 2
    with TileContext(nc) as tc:
        with (
            tc.tile_pool(name="wpool", bufs=1) as wpool,
            tc.tile_pool(name="xpool", bufs=6) as xpool,
            tc.tile_pool(name="hpool", bufs=10) as hpool,
            tc.tile_pool(name="opool", bufs=4) as opool,
            tc.tile_pool(name="psum", bufs=4, space="PSUM") as psum_pool,
        ):
            w0_sb = wpool.tile([2 * N_STREAMS, 128], mm_dt)
            nc.sync.dma_start(out=w0_sb, in_=w0[:, :])
            b0_sb = wpool.tile([128, 1], f32)
            nc.sync.dma_start(out=b0_sb, in_=b0[:, :])
            k2_sb = wpool.tile([128, 1], f32)
            nc.sync.dma_start(out=k2_sb, in_=k2c[:, :])
            wh_sb = []
            for l in range(N_HID_LAYERS):
                w_l = wpool.tile([128, 128], mm_dt, name=f"wh{l}")
                nc.sync.dma_start(out=w_l, in_=wh[l, :, :])
                wh_sb.append(w_l)
            wo_sb = wpool.tile([128, 3 * N_STREAMS], mm_dt)
            nc.sync.dma_start(out=wo_sb, in_=wo[:, :])

            # per-layer unit = half a chunk ([128, 1024] PSUM slot)
            ORDER = tuple(range(GRP))
            SPLIT_L = 3          # hidden layer split between ACT and DVE
            ACT_CHUNKS = (0,)    # chunks of SPLIT_L handled by ACT

            def unit(ps, w_sb, h_in, hb):
                for k in range(2):
                    bs = slice(hb * half + k * mm_block,
                               hb * half + (k + 1) * mm_block)
                    nc.tensor.matmul(ps[:, k * mm_block:(k + 1) * mm_block],
                                     w_sb, h_in[:, bs], start=True, stop=True)

            # software-pipelined state
            x_sb = {}   # (q, i) -> x tile
            h_cur = {}  # (q, i) -> latest h tile

            def emit_x(q):
                for i in range(GRP):
                    cs = slice((GRP * q + i) * chunk, (GRP * q + i + 1) * chunk)
                    t = xpool.tile([2 * N_STREAMS, chunk], mm_dt, tag="x",
                                   name=f"x{i}")
                    nc.sync.dma_start(out=t, in_=xT[:, cs])
                    x_sb[(q, i)] = t

            def hidden_unit(q, l, i, hb, h_next, dve):
                ps = psum_pool.tile([128, half], f32, tag="ps",
                                    name=f"ps{i}{hb}")
                unit(ps, wh_sb[l], h_cur[(q, i)], hb)
                dst = h_next[:, hb * half:(hb + 1) * half]
                if dve:
                    nc.vector._custom_dve(
                        TANH_OP, out=dst, in0=ps[:, :], in1=k2_sb[:, :],
                        s0=KS[l][0], s1=KS[l][1], imm2=BC)
                else:
                    nc.scalar.activation(dst, ps, AF.Tanh)

            def gen_head(q):
                # input layer (bias) on ACT
                for i in ORDER:
                    t = hpool.tile([128, chunk], mm_dt, tag="h", name=f"h{i}")
                    for hb in range(2):
                        ps = psum_pool.tile([128, half], f32, tag="ps",
                                            name=f"ps{i}{hb}")
                        unit(ps, w0_sb, x_sb.pop((q, i)) if hb else
                             x_sb[(q, i)], hb)
                        nc.scalar.activation(
                            t[:, hb * half:(hb + 1) * half], ps,
                            AF.Tanh, bias=b0_sb)
                        h_cur[(q, i)] = t
                        yield
                # hidden layers 0..SPLIT_L-1 on ACT (chunk-split on the
                # first group, where there is no concurrent tail for DVE)
                for l in range(SPLIT_L):
                    for i in ORDER:
                        t = hpool.tile([128, chunk], mm_dt, tag="h",
                                       name=f"h{i}")
                        for hb in range(2):
                            hidden_unit(q, l, i, hb, t,
                                        dve=(q == 0 and i == 1))
                            yield
                        h_cur[(q, i)] = t
                # SPLIT_L for ACT's chunks
                for i in ACT_CHUNKS:
                    t = hpool.tile([128, chunk], mm_dt, tag="h", name=f"h{i}")
                    for hb in range(2):
                        hidden_unit(q, SPLIT_L, i, hb, t, dve=False)
                        yield
                    h_cur[(q, i)] = t

            def gen_tail(q):
                # SPLIT_L for DVE's chunks
                for i in ORDER:
                    if i in ACT_CHUNKS:
                        continue
                    t = hpool.tile([128, chunk], mm_dt, tag="h", name=f"h{i}")
                    for hb in range(2):
                        hidden_unit(q, SPLIT_L, i, hb, t, dve=True)
                        yield
                    h_cur[(q, i)] = t
                # hidden layers SPLIT_L+1..7 on DVE (chunk-split on the
                # last group, where there is no concurrent head for ACT)
                for l in range(SPLIT_L + 1, N_HID_LAYERS):
                    for i in ORDER:
                        t = hpool.tile([128, chunk], mm_dt, tag="h",
                                       name=f"h{i}")
                        for hb in range(2):
                            hidden_unit(q, l, i, hb, t,
                                        dve=not (q == n_grp - 1 and i == 0))
                            yield
                        h_cur[(q, i)] = t

            def gen_out(q):
                o_grp = opool.tile([128, GRP * mm_block], f32, tag="o",
                                   name="o")
                for pair in range(GRP // 2):
                    ps_o = psum_pool.tile([128, half], f32, tag="ps",
                                          name=f"pso{pair}")
                    for ii in range(2):
                        i = 2 * pair + ii
                        hq = h_cur[(q, i)]
                        for k in range(blocks):
                            bs = slice(k * mm_block, (k + 1) * mm_block)
                            os_ = slice(ii * mm_block, (ii + 1) * mm_block)
                            nc.tensor.matmul(
                                ps_o[32 * k:32 * k + 3 * N_STREAMS, os_],
                                wo_sb, hq[:, bs], start=True, stop=True,
                                tile_position=(0, 32 * k))
                        yield
                    nc.scalar.activation(
                        o_grp[:, pair * half:(pair + 1) * half], ps_o,
                        AF.Sigmoid)
                    yield
                for i in range(GRP):
                    del h_cur[(q, i)]
                # o_grp[32k+m, 512i+c] = out[m, (GRP*q+i)*2048 + 512k + c]
                for k in range(blocks):
                    src = o_grp[32 * k:32 * k + 3 * N_STREAMS, :]
                    dst = out.rearrange("m (q i kk c) -> kk m q i c",
                                        q=n_grp, i=GRP, kk=blocks)[k, :, q]
                    nc.sync.dma_start(out=dst, in_=src)
                yield

            emit_x(0)
            for q in range(n_grp + 2):
                if q + 1 < n_grp:
                    emit_x(q + 1)
                # weave tail(q-1) [DVE], head(q) [ACT] and out(q-2)
                # [PE+sigmoid] so no engine starves at group boundaries
                tail_it = gen_tail(q - 1) if 1 <= q < n_grp + 1 else iter(())
                head_it = gen_head(q) if q < n_grp else iter(())
                out_it = gen_out(q - 2) if 2 <= q < n_grp + 2 else iter(())
                weave = itertools.cycle((tail_it, head_it, tail_it, head_it,
                                         out_it))
                live = {id(tail_it): True, id(head_it): True, id(out_it): True}
                done = 0
                while done < 3:
                    it = next(weave)
                    if not live[id(it)]:
                        continue
                    if next(it, StopIteration) is StopIteration:
                        live[id(it)] = False
                        done = sum(1 for v in live.values() if not v)

    nc.compile()
    _GRAPH_CACHE[key] = nc
    return nc


def _pack_weights(W0, b0, Wh, Wout):
    w0p = np.zeros((2 * N_STREAMS, 128), np.float32)
    b0p = np.zeros((128, 1), np.float32)
    whp = np.zeros((N_HID_LAYERS, 128, 128), np.float32)
    wop = np.zeros((128, 3 * N_STREAMS), np.float32)
    for s in range(N_STREAMS):
        w0p[2 * s:2 * s + 2, 16 * s:16 * s + 16] = W0.T
        b0p[16 * s:16 * s + 16, 0] = b0
        for l in range(N_HID_LAYERS):
            whp[l, 16 * s:16 * s + 16, 16 * s:16 * s + 16] = Wh[l].T
        wop[16 * s:16 * s + 16, 3 * s:3 * s + 3] = Wout.T
    return w0p, b0p, whp, wop


def kernel(x, W0, b0, Wh, Wout, trace=False):
    x = np.asarray(x, np.float32)
    W0 = np.asarray(W0, np.float32)
    b0 = np.asarray(b0, np.float32)
    Wh = np.asarray(Wh, np.float32)
    Wout = np.asarray(Wout, np.float32)

    nc = _build_graph(N_COLS)
    w0p, b0p, whp, wop = _pack_weights(W0, b0, Wh, Wout)
    w0p, whp, wop = (a.astype(BF16) for a in (w0p, whp, wop))
    k2arr = np.full((128, 1), K2, np.float32)

    per_core = N_STREAMS * N_COLS
    in_maps = []
    for c in range(N_CORES):
        xs = x[c * per_core:(c + 1) * per_core]
        xT = np.ascontiguousarray(
            xs.reshape(N_STREAMS, N_COLS, 2).transpose(0, 2, 1)
        ).reshape(2 * N_STREAMS, N_COLS)
        in_maps.append({"xT": xT.astype(BF16), "w0": w0p, "b0": b0p,
                        "wh": whp, "wo": wop, "k2c": k2arr})

    res = run_bass_kernel_spmd(nc, in_maps, core_ids=list(range(N_CORES)),
                               trace=trace)
    LAST_RUN_INFO.clear()
    LAST_RUN_INFO["exec_time_ns"] = res.exec_time_ns
    prof = getattr(res, "instructions_and_trace", None)
    LAST_RUN_INFO["trace_dir"] = getattr(prof, "trace_dir", None)

    parts = []
    for r in res.results:
        o = r["out"].reshape(N_STREAMS, 3, N_COLS).transpose(0, 2, 1)
        parts.append(o.reshape(per_core, 3))
    return np.concatenate(parts, axis=0)
